# revision 11
# baseline (speedup 1.0000x reference)
"""GAT-style edge-softmax self-attention, dense-mask formulation, 8-core SPMD.

Math: per batch b (one NeuronCore per batch),
  Q/K/V = X @ Wq/k/v ; per head h: S = Q_h K_h^T / 8
  ex = C * exp(S)           (C[i,j] = multiplicity of edge (i<-j); softmax is
                             shift-invariant and |S| <~ 7, so no row-max needed)
  numerator_i = (ex @ V)_i ; denominator_i = sum_j ex_ij
The denominator comes from a ones-column appended to V per head; the device
returns numerator/denominator stacked transposed ([12*65, 1024]) and the host
performs the final divide + transpose.

Schedule: V-projection chunks are issued just-in-time inside head 0's
j-loop; each feature chunk's Q/K projection is interleaved with the
attention of its two heads. This keeps the PE stream dense (HAM stays
un-throttled at 2.4 GHz) and the scalar engine exclusively runs exp —
the critical path — from early in the kernel.
"""

import numpy as np
import ml_dtypes

import concourse.bass as bass
import concourse.bacc as bacc
import concourse.mybir as mybir
import concourse.tile as tile
from concourse.bass_utils import run_bass_kernel_spmd

B, N, H = 8, 1024, 768
NH, HD = 12, 64
P = 128
KC = H // P   # 6 contraction chunks for projections
JC = N // P   # 8 node chunks
F32 = mybir.dt.float32
BF16 = mybir.dt.bfloat16

_CACHE = {}


def _build_nc():
    nc = bacc.Bacc("TRN2", target_bir_lowering=False, debug=True)

    xT_d = nc.dram_tensor("xT", [H, N], BF16, kind="ExternalInput")
    wq_d = nc.dram_tensor("wq", [H, H], BF16, kind="ExternalInput")
    wk_d = nc.dram_tensor("wk", [H, H], BF16, kind="ExternalInput")
    wv_d = nc.dram_tensor("wv", [H, H], BF16, kind="ExternalInput")
    mT_d = nc.dram_tensor("maskT", [N, N], BF16, kind="ExternalInput")
    # numerator+denominator, transposed: row h*65+d = V-dim d of head h,
    # row h*65+64 = denominator of head h; columns = head-node index i.
    oT_d = nc.dram_tensor("outT", [NH * (HD + 1), N], F32, kind="ExternalOutput")

    with tile.TileContext(nc) as tc:
        with tc.tile_pool(name="res", bufs=1) as res, \
             tc.tile_pool(name="work", bufs=4) as work, \
             tc.tile_pool(name="sps", bufs=2, space="PSUM") as spsp, \
             tc.tile_pool(name="avs", bufs=2, space="PSUM") as avsp:

            # ---- resident loads (order = need order) ----
            xT = [res.tile([P, N], BF16, tag=f"xT{k}", name=f"xT{k}") for k in range(KC)]
            wq = [res.tile([P, H], BF16, tag=f"wq{k}", name=f"wq{k}") for k in range(KC)]
            wk = [res.tile([P, H], BF16, tag=f"wk{k}", name=f"wk{k}") for k in range(KC)]
            wv = [res.tile([P, H], BF16, tag=f"wv{k}", name=f"wv{k}") for k in range(KC)]
            mT = [res.tile([P, N], BF16, tag=f"mT{j}", name=f"mT{j}") for j in range(JC)]
            for k in range(KC):
                nc.default_dma_engine.dma_start(out=xT[k][:], in_=xT_d[k * P:(k + 1) * P, :])
                nc.default_dma_engine.dma_start(out=wv[k][:], in_=wv_d[k * P:(k + 1) * P, :])
                nc.default_dma_engine.dma_start(out=wq[k][:], in_=wq_d[k * P:(k + 1) * P, :])
                nc.default_dma_engine.dma_start(out=wk[k][:], in_=wk_d[k * P:(k + 1) * P, :])
            for j in range(JC):
                nc.default_dma_engine.dma_start(out=mT[j][:], in_=mT_d[j * P:(j + 1) * P, :])

            # computed residents
            qT = [res.tile([P, N], BF16, tag=f"qT{k}", name=f"qT{k}") for k in range(KC)]
            kT = [res.tile([P, N], BF16, tag=f"kT{k}", name=f"kT{k}") for k in range(KC)]
            # V packed per head with a trailing ones column: cols h*65..h*65+63
            # hold V_h, col h*65+64 holds 1.0 (denominator trick).
            vp = [res.tile([P, NH * (HD + 1)], BF16, tag=f"vp{j}", name=f"vp{j}") for j in range(JC)]
            for j in range(JC):
                nc.gpsimd.memset(vp[j][:], 1.0)

            def v_proj(j):
                # both 512/256 segments into one scores-ring tile (each
                # segment stays inside one PSUM bank).
                ps = spsp.tile([P, N], F32, tag="s")
                for nn, (c0, cw) in enumerate(((0, 512), (512, 256))):
                    for k in range(KC):
                        nc.tensor.matmul(
                            ps[:, nn * 512:nn * 512 + cw],
                            xT[k][:, j * P:(j + 1) * P],
                            wv[k][:, c0:c0 + cw],
                            start=(k == 0), stop=(k == KC - 1),
                        )
                for h in range(NH):
                    for nn, (c0, cw) in enumerate(((0, 512), (512, 256))):
                        lo = max(h * HD, c0)
                        hi = min((h + 1) * HD, c0 + cw)
                        if lo >= hi:
                            continue
                        nc.vector.tensor_copy(
                            out=vp[j][:, h * (HD + 1) + (lo - h * HD):
                                      h * (HD + 1) + (hi - h * HD)],
                            in_=ps[:, nn * 512 + lo - c0:nn * 512 + hi - c0])

            def qk_proj(c6):
                # q copy on vector, k copy on scalar: keeps exp mostly
                # unblocked while spreading cast work.
                for w_sb, dst, eng in ((wq, qT, "v"), (wk, kT, "s")):
                    ps = spsp.tile([P, N], F32, tag="s")
                    for nn in range(2):
                        for k in range(KC):
                            nc.tensor.matmul(
                                ps[:, nn * 512:(nn + 1) * 512],
                                w_sb[k][:, c6 * P:(c6 + 1) * P],
                                xT[k][:, nn * 512:(nn + 1) * 512],
                                start=(k == 0), stop=(k == KC - 1),
                            )
                    if eng == "v":
                        nc.vector.tensor_copy(out=dst[c6][:], in_=ps[:])
                    else:
                        nc.scalar.copy(out=dst[c6][:], in_=ps[:])

            def head(h, with_v=False):
                c6 = h // 2
                kt, qt = kT[c6], qT[c6]
                off = (h % 2) * HD
                av = avsp.tile([P, N], F32, tag="av")
                for j in range(JC):
                    if with_v:
                        v_proj(j)
                    sps = spsp.tile([P, N], F32, tag="s")
                    for nn in range(2):
                        nc.tensor.matmul(
                            sps[:, nn * 512:(nn + 1) * 512],
                            kt[off:off + HD, j * P:(j + 1) * P],
                            qt[off:off + HD, nn * 512:(nn + 1) * 512],
                            start=True, stop=True,
                        )
                    exf = work.tile([P, N], BF16, tag="exf")
                    nc.scalar.activation(
                        exf[:], sps[:],
                        mybir.ActivationFunctionType.Exp, scale=0.125)
                    exm = work.tile([P, N], BF16, tag="exm")
                    nc.vector.tensor_tensor(
                        out=exm[:], in0=exf[:], in1=mT[j][:],
                        op=mybir.AluOpType.mult)
                    for nn in range(2):
                        nc.tensor.matmul(
                            av[0:HD + 1, nn * 512:(nn + 1) * 512],
                            vp[j][:, h * (HD + 1):(h + 1) * (HD + 1)],
                            exm[:, nn * 512:(nn + 1) * 512],
                            start=(j == 0), stop=(j == JC - 1),
                        )
                av_sb = work.tile([HD + 1, N], F32, tag="avsb")
                nc.vector.tensor_copy(out=av_sb[:], in_=av[0:HD + 1, :])
                nc.default_dma_engine.dma_start(
                    out=oT_d[h * (HD + 1):(h + 1) * (HD + 1), :],
                    in_=av_sb[:])

            qk_proj(0)
            head(0, with_v=True)
            head(1)
            for c6 in range(1, KC):
                qk_proj(c6)
                head(2 * c6)
                head(2 * c6 + 1)

    nc.compile()
    return nc


def _make_in_maps(node_states, edge_indices, Wq, Wk, Wv):
    eb, ei, ej = (np.asarray(edge_indices[r]) for r in range(3))
    bf = ml_dtypes.bfloat16
    CT = np.zeros((B, N, N), dtype=np.float32)
    np.add.at(CT, (eb, ej, ei), 1.0)  # CT[b, j, i] = multiplicity of edge (i<-j)

    wq = np.ascontiguousarray(Wq).astype(bf)
    wk = np.ascontiguousarray(Wk).astype(bf)
    wv = np.ascontiguousarray(Wv).astype(bf)

    in_maps = []
    for b in range(B):
        in_maps.append({
            "xT": np.ascontiguousarray(np.asarray(node_states[b]).T).astype(bf),
            "wq": wq, "wk": wk, "wv": wv,
            "maskT": CT[b].astype(bf),
        })
    return in_maps


def kernel(node_states, edge_indices, Wq, Wk, Wv):
    if "nc" not in _CACHE:
        _CACHE["nc"] = _build_nc()
    nc = _CACHE["nc"]

    in_maps = _make_in_maps(node_states, edge_indices, Wq, Wk, Wv)
    res = run_bass_kernel_spmd(nc, in_maps, list(range(B)))
    out = np.empty((B, N, H), dtype=np.float32)
    for b in range(B):
        r = np.asarray(res.results[b]["outT"]).reshape(NH, HD + 1, N)
        den = np.maximum(r[:, HD, :], 1e-9)          # (NH, N)
        num = r[:, :HD, :] / den[:, None, :]         # (NH, HD, N)
        out[b] = num.transpose(2, 0, 1).reshape(N, H)
    return out
